# revision 57
# baseline (speedup 1.0000x reference)
"""Trainium2 Bass kernel for nn_GAT_38989713113447 (3-layer dense GAT).

Sharding: 8 heads over 8 cores for the two inner GAT layers (pure head
parallelism, no communication).  The head-concat + output projection
commutes into a sum of per-head projections: Who = sum_k h1_k @ Wo[k],
and EVERYTHING the output layer consumes is linear in Who, so each core
pre-stages its head's contribution to (a) node-major Who blocks
[24][128, 64] and (b) the f1/f2 logit columns (via host-folded
wo1 = Wo@ao1, wo2 = Wo@ao2) into one packed [66, 3072] buffer that a
single AllReduce(add) sums.  Post-collective work is just two coalesced
DMA loads + exp/scale + the masked output attention.  The output
attention is sharded over node rows (384 rows/core); the per-core
column slice of f1 is selected via a one-hot matmul against a per-core
psel input (keeps the SPMD program identical on every core).  The
final [3072, 64] output is assembled host-side from per-core slices.

Attention math (per column i, softmax over j):  the per-column factor
exp(f1_i) cancels in softmax, so with
    at[j,i]  = m * max(exp(f1_i+f2_j), exp(a*f1_i + a*f2_j))      (a=0.2)
    at'[j,i] = at[j,i] / exp(f1_i)
             = m * max(exp(f2_j), exp((a-1)*f1_i + a*f2_j))
the first branch is per-PARTITION constant v_j = exp(f2_j).  One ACT
pass builds e1' = Exp((a-1)*f1bc + a*f2_j) (bias per-partition), one
DVE scalar_tensor_tensor builds at' = (e1' max v_j) * m.  That is 1 ACT
+ 1 DVE pass over [N,N] per layer (the unfactored form needs 2 DVE).

mergeState (x + seed*theta) is folded into the hT input host-side.

dtypes: masks and at' are fp8e4 (values O(1), exact {0,1} masks); the
adjacency mask is DMA'd once (on the idle Pool queue) and stays
SBUF-resident across all three layers.  Aggregation matmuls run mixed
lhsT=bf16(Wh) x rhs=fp8(at'); softmax denominators use fp8 DoubleRow
matmuls (2 j-tiles per pass) against an interleaved at-pair layout.
"""

import os
import sys

sys.path.insert(0, "/opt/trn_rl_repo")

from contextlib import ExitStack

import numpy as np
import ml_dtypes

import concourse.bass as bass  # noqa: F401
import concourse.bacc as bacc
import concourse.tile as tile
from concourse import mybir
from concourse.bass_utils import run_bass_kernel_spmd

N = 3072
F = 256
H = 8
D = 128          # H1 == H2
OUT = 64
ALPHA = 0.2
NCORES = 8
NJB = N // 128   # 24 attention j-blocks
HALF = N // 2    # i-dim half per PSUM residency
ISL = N // NCORES  # 384 output rows per core

FP32 = mybir.dt.float32
BF16 = mybir.dt.bfloat16
FP8 = mybir.dt.float8e4
AF = mybir.ActivationFunctionType
ALU = mybir.AluOpType

def _chunks(total, step):
    return [(o, min(step, total - o)) for o in range(0, total, step)]


class Builder:
    def __init__(self, nc, tc, ctx):
        self.nc = nc
        self.tc = tc
        p = lambda name, bufs, space=None: ctx.enter_context(
            tc.tile_pool(name=name, bufs=bufs, **({"space": space} if space else {}))
        )
        self.state = p("state", 1)
        self.mres = p("mres", 1)
        self.work = p("work", 4)
        self.att = p("att", 4)
        self.ps_agg = p("ps_agg", 1, "PSUM")
        self.ps_rs = p("ps_rs", 1, "PSUM")
        self.ps_sm = p("ps_sm", 2, "PSUM")
        self.misc = p("misc", 1)

    def ones_tile(self, shape, dtype, name):
        t = self.state.tile(shape, dtype, tag=name, name=name)
        self.nc.vector.memset(t[:, :], 1.0)
        return t

    def ones_pair(self):
        """fp8 [128, 2, 1] stationary AP for DoubleRow rowsum (cols 0, 16)."""
        t = self.state.tile([128, 32], FP8, tag="ones_pair", name="ones_pair")
        nc = self.nc
        nc.vector.memset(t[:, :], 0.0)
        nc.vector.memset(t[:, 0:1], 1.0)
        nc.vector.memset(t[:, 16:17], 1.0)
        ap = t[:, :]
        return bass.AP(ap.tensor, ap.offset,
                       [list(ap.ap[0])] + [[16, 2], [1, 1]])

    def _rowd(self, tag, rows=1):
        if not hasattr(self, "_row_dram"):
            self._row_dram = {}
        if tag not in self._row_dram:
            self._row_dram[tag] = self.nc.dram_tensor(
                f"rowd_{tag}", [rows, N], BF16)
        return self._row_dram[tag]

    def bcast_row(self, row_ap, width, tag):
        """[1, width] bf16 SBUF row -> [128, width] bf16 tile via a DMA with
        a partition-step-0 source AP (reads the row 128x).  Issued on the
        Activation HWDGE queue: these sit at layer boundaries and would
        head-of-line block the sync or Pool queues."""
        nc = self.nc
        rd = self._rowd(tag)
        out = self.state.tile([128, width], BF16, tag=tag)
        for off, w in _chunks(width, 512):
            nc.scalar.dma_start(rd[:, off : off + w],
                                row_ap[:, off : off + w])
            rs_ap = rd[:1, off : off + w]
            bsrc = bass.AP(rs_ap.tensor, rs_ap.offset,
                           [[0, 128]] + [list(p) for p in rs_ap.ap])
            nc.scalar.dma_start(out[:, off : off + w], bsrc)
        return out

    def project(self, hT_tiles, w_sb, dT, jb_range=None, tagp="whnm",
                pools=None, copy_dve=False):
        """Node-major Wh_nm[jb][j, dT] = (h @ W) blocks.  pools: optional
        list of (pool, tag) to rotate PSUM tiles through (the attention
        pools are free outside the attention loops -> deeper buffering).
        copy_dve: do PSUM->SBUF copies on DVE (idle at startup) so the ACT
        queue stays clear for the f1row/f1bc chain."""
        nc = self.nc
        nk = len(hT_tiles)
        wh_nm = []
        per = max(1, 512 // dT)  # jb blocks packed per PSUM tile
        jlo, jhi = jb_range if jb_range is not None else (0, NJB)
        for gi, jb0 in enumerate(range(jlo, jhi, per)):
            njb = min(per, jhi - jb0)
            if pools:
                pool, ptag = pools[gi % len(pools)]
                ps = pool.tile([128, 512], FP32, tag=ptag, name=ptag)
            else:
                ps = self.ps_sm.tile([128, 512], FP32, tag="sm", name="sm")
            for u in range(njb):
                jb = jb0 + u
                for k in range(nk):
                    nc.tensor.matmul(
                        ps[:, u * dT : (u + 1) * dT],
                        hT_tiles[k][:, jb * 128 : (jb + 1) * 128],
                        w_sb[k][:, :],
                        start=(k == 0), stop=(k == nk - 1),
                    )
            t = self.state.tile([128, per * dT], BF16,
                                tag=f"{tagp}_{jb0 // per}", name="whnm")
            if copy_dve:
                nc.vector.tensor_copy(t[:, : njb * dT], ps[:, : njb * dT])
            else:
                nc.scalar.activation(t[:, : njb * dT], ps[:, : njb * dT],
                                     AF.Copy)
            for u in range(njb):
                wh_nm.append(t[:, u * dT : (u + 1) * dT])
        return wh_nm

    def f2_cols_into(self, vcol, af2col, hT_tiles, wa2_sb, jlo, jhi):
        """Columns [jlo, jhi) of vcol = exp(f2col), af2col = ALPHA*f2col.
        f2[jb] depends only on h columns of block jb, so the first half is
        emittable mid-attention (between i-halves)."""
        nc = self.nc
        nk = len(hT_tiles)
        nb = jhi - jlo
        f2ps = self.ps_sm.tile([128, NJB], FP32, tag="sm", name="sm")
        for u, jb in enumerate(range(jlo, jhi)):
            for k in range(nk):
                nc.tensor.matmul(
                    f2ps[:, u : u + 1],
                    hT_tiles[k][:, jb * 128 : (jb + 1) * 128],
                    wa2_sb[k][:, :],
                    start=(k == 0), stop=(k == nk - 1),
                )
        nc.scalar.activation(vcol[:, jlo:jhi], f2ps[:, :nb], AF.Exp)
        nc.scalar.activation(af2col[:, jlo:jhi], f2ps[:, :nb], AF.Copy,
                             scale=ALPHA)

    def fcols_tiles(self, tag):
        vcol = self.state.tile([128, NJB], FP32, tag=tag + "v")
        af2col = self.state.tile([128, NJB], FP32, tag=tag + "a")
        return vcol, af2col

    def f1_part(self, hT_tiles, wa1_sb, f1row, f1bc, rowd_tag, lo, hi,
                pre_attention=False):
        """f1row/f1bc columns [lo, hi): row matmuls + broadcast DMAs."""
        nc = self.nc
        nk = len(hT_tiles)
        rd = self._rowd(rowd_tag)
        for off, w in _chunks(hi - lo, 512):
            o = lo + off
            ps = self.ps_sm.tile([1, 512], FP32, tag="sm", name="sm")
            for k in range(nk):
                nc.tensor.matmul(
                    ps[:, :w], wa1_sb[k][:, :],
                    hT_tiles[k][:, o : o + w],
                    start=(k == 0), stop=(k == nk - 1),
                )
            nc.scalar.activation(f1row[:, o : o + w], ps[:, :w], AF.Copy)
            if pre_attention:
                # the rowsum PSUM banks are free pre-attention: broadcast
                # via a PE ones-matmul + DVE copy (no DMA latency/contention)
                bp = self.ps_rs.tile([128, 512], FP32,
                                     tag=f"rs{(o // 512) % 3}", name="rs")
                nc.tensor.matmul(bp[:, :w], self.ones1b[:, :128],
                                 f1row[:, o : o + w], start=True, stop=True)
                nc.vector.tensor_copy(f1bc[:, o : o + w], bp[:, :w])
            else:
                # sync queue: mid-attention the ACT queue is saturated with
                # e1 ops and a waiting DMA would head-of-line block them
                nc.sync.dma_start(rd[:, o : o + w], f1row[:, o : o + w])
        if not pre_attention:
            # reads in a second pass so the write->read pairs pipeline
            for off, w in _chunks(hi - lo, 512):
                o = lo + off
                rs_ap = rd[:1, o : o + w]
                bsrc = bass.AP(rs_ap.tensor, rs_ap.offset,
                               [[0, 128]] + [list(p) for p in rs_ap.ap])
                nc.sync.dma_start(f1bc[:, o : o + w], bsrc)

    def attention_agg(self, mres, mcol, vcol, af2col, f1bc, wh_nm, dT,
                      width, h_out, out_elu, between=None):
        """Factored masked softmax + aggregation + normalize (+ELU).

        mres: list of NJB resident fp8 mask tiles; mcol: column offset into
        them.  between: callback after the first i-half's emission (its
        inputs must be fully emitted by then)."""
        nc = self.nc
        half_w = min(width, HALF)
        for h0 in range(0, width, half_w):
            if h0 > 0 and between is not None:
                between()
                between = None
            hw = min(half_w, width - h0)
            ch = _chunks(hw, 512)
            agg_ps = [self.ps_agg.tile([dT, 512], FP32, tag=f"agg{ci}",
                                       name=f"agg{ci}")
                      for ci in range(len(ch))]
            rs_ps = [self.ps_rs.tile([1, 512], FP32, tag=f"rs{ci}",
                                     name=f"rs{ci}")
                     for ci in range(len(ch))]
            npair = NJB // 2
            n_gp = int(os.environ.get("GAT_GP_STT", "0"))
            for pair in range(npair):
                atp = self.att.tile([128, 2 * hw], FP8, tag="atp")
                on_gp = n_gp and (pair % (npair // max(n_gp, 1)) ==
                                  npair // max(n_gp, 1) - 1)
                eng = nc.gpsimd if on_gp else nc.vector
                for side in range(2):
                    jb = 2 * pair + side
                    e1 = self.work.tile([128, hw], FP8, tag="e1")
                    nc.scalar.activation(
                        e1[:, :], f1bc[:, h0 : h0 + hw], AF.Exp,
                        scale=ALPHA - 1.0, bias=af2col[:, jb : jb + 1],
                    )
                    eng.scalar_tensor_tensor(
                        atp[:, side * hw : (side + 1) * hw], e1[:, :],
                        vcol[:, jb : jb + 1],
                        mres[jb][:, mcol + h0 : mcol + h0 + hw],
                        ALU.max, ALU.mult,
                    )
                for side in range(2):
                    jb = 2 * pair + side
                    for ci, (off, w) in enumerate(ch):
                        nc.tensor.matmul(
                            agg_ps[ci][:, :w], wh_nm[jb],
                            atp[:, side * hw + off : side * hw + off + w],
                            start=(jb == 0), stop=(jb == NJB - 1),
                        )
                for ci, (off, w) in enumerate(ch):
                    base = atp[:, off : off + w]
                    rhs = bass.AP(base.tensor, base.offset,
                                  [list(base.ap[0])] + [[hw, 2], [1, w]])
                    nc.tensor.matmul(
                        rs_ps[ci][:, :w], self.ones_pair_ap, rhs,
                        start=(pair == 0), stop=(pair == npair - 1),
                        perf_mode=mybir.MatmulPerfMode.DoubleRow,
                    )
            # softmax denominator -> reciprocal -> broadcast -> normalize
            rinv = self.misc.tile([1, hw], FP32, tag="rinv")
            for ci, (off, w) in enumerate(ch):
                nc.vector.reciprocal_approx_fast(rinv[:, off : off + w],
                                                 rs_ps[ci][:, :w])
            rb_sb = self.misc.tile([dT, hw], BF16, tag="rb_sb")
            for off, w in _chunks(hw, 512):
                ps = self.ps_sm.tile([dT, 512], FP32, tag="sm", name="sm")
                nc.tensor.matmul(
                    ps[:, :w], self.ones1f[:, :dT], rinv[:, off : off + w],
                    start=True, stop=True,
                )
                nc.scalar.activation(rb_sb[:, off : off + w], ps[:, :w], AF.Copy)
            hpn = self.misc.tile([dT, hw], FP32 if not out_elu else BF16,
                                 tag="hpn")
            # stage agg out of PSUM via ACT (has slack) so the normalize
            # multiply runs on DVE (the binding engine) at 2x_1p bf16
            # instead of 1x with the PSUM access penalty
            asb = self.misc.tile([dT, hw], BF16, tag="asb")
            for ci, (off, w) in enumerate(ch):
                if out_elu:
                    nc.scalar.activation(asb[:, off : off + w],
                                         agg_ps[ci][:dT, :w], AF.Copy)
                    nc.vector.tensor_tensor(
                        hpn[:, off : off + w], asb[:, off : off + w],
                        rb_sb[:, off : off + w], ALU.mult,
                    )
                else:
                    nc.vector.tensor_tensor(
                        hpn[:, off : off + w], agg_ps[ci][:dT, :w],
                        rb_sb[:, off : off + w], ALU.mult,
                    )
            if out_elu:
                # ELU(x) = exp(min(x,0)) - 1 + max(x,0)
                m = self.misc.tile([dT, hw], BF16, tag="elu_m")
                nc.vector.tensor_scalar(m[:, :], hpn[:, :], 0.0, None, ALU.min)
                e = self.misc.tile([dT, hw], BF16, tag="elu_e")
                nc.scalar.activation(e[:, :], m[:, :], AF.Exp)
                r = self.misc.tile([dT, hw], BF16, tag="elu_m")
                nc.vector.tensor_scalar(
                    r[:, :], hpn[:, :], 0.0, -1.0, ALU.max, ALU.add
                )
                nc.vector.tensor_add(h_out[:, h0 : h0 + hw], e[:, :], r[:, :])
            else:
                nc.vector.tensor_copy(h_out[:, h0 : h0 + hw], hpn[:, :])


def build(dbg=False, sim=False):
    nc = bacc.Bacc("TRN2", target_bir_lowering=False, num_devices=NCORES)

    xT = nc.dram_tensor("xT", [F, N], BF16, kind="ExternalInput")
    adjT = nc.dram_tensor("adjT", [N, N], FP8, kind="ExternalInput")
    adjT_osl = nc.dram_tensor("adjT_osl", [N, ISL], FP8, kind="ExternalInput")
    w0 = nc.dram_tensor("w0", [F, D], BF16, kind="ExternalInput")
    a01 = nc.dram_tensor("a01", [F, 1], BF16, kind="ExternalInput")
    a02 = nc.dram_tensor("a02", [F, 1], BF16, kind="ExternalInput")
    w1 = nc.dram_tensor("w1", [D, D], BF16, kind="ExternalInput")
    a11 = nc.dram_tensor("a11", [D, 1], BF16, kind="ExternalInput")
    a12 = nc.dram_tensor("a12", [D, 1], BF16, kind="ExternalInput")
    wo = nc.dram_tensor("wo", [D, OUT], BF16, kind="ExternalInput")
    wo1 = nc.dram_tensor("wo1", [D, 1], BF16, kind="ExternalInput")
    wo2 = nc.dram_tensor("wo2", [D, 1], BF16, kind="ExternalInput")
    psel = nc.dram_tensor("psel", [NJB // 2 * 128, 2 * ISL], FP8,
                          kind="ExternalInput")

    outT = nc.dram_tensor("outT", [OUT, ISL], FP32, kind="ExternalOutput")
    if dbg:
        h0_dbg = nc.dram_tensor("h0_dbg", [D, N], BF16, kind="ExternalOutput")
        h1_dbg = nc.dram_tensor("h1_dbg", [D, N], BF16, kind="ExternalOutput")

    ar_in = nc.dram_tensor("ar_in", [OUT + 2, N], BF16)
    ar_out = nc.dram_tensor("ar_out", [OUT + 2, N], BF16, addr_space="Shared")
    REG_NM, REG_F2, REG_F1 = 0, OUT * N, OUT * N + N

    with tile.TileContext(nc) as tc, ExitStack() as ctx:
        b = Builder(nc, tc, ctx)
        b.ones1f = b.ones_tile([1, 128], FP32, "ones1f")
        b.ones1b = b.ones_tile([1, 128], BF16, "ones1b")
        b.ones_pair_ap = b.ones_pair()

        def load_w(ap, shape, tag, dt=BF16):
            s = b.state.tile(shape, dt, tag=tag, name=tag)
            nc.sync.dma_start(s[:, :], ap)
            return s

        w0_sb = [load_w(w0[k * 128 : (k + 1) * 128, :], [128, D], f"w0_{k}")
                 for k in range(F // 128)]
        wa01_sb = [load_w(a01[k * 128 : (k + 1) * 128, :], [128, 1],
                          f"wa01_{k}") for k in range(F // 128)]
        wa02_sb = [load_w(a02[k * 128 : (k + 1) * 128, :], [128, 1],
                          f"wa02_{k}") for k in range(F // 128)]
        w1_sb = [load_w(w1[:, :], [D, D], "w1")]
        wa11_sb = [load_w(a11[:, :], [D, 1], "wa11")]
        wa12_sb = [load_w(a12[:, :], [D, 1], "wa12")]
        wo_sb = [load_w(wo[:, :], [D, OUT], "wo")]
        wo1_sb = [load_w(wo1[:, :], [D, 1], "wo1")]
        wo2_sb = [load_w(wo2[:, :], [D, 1], "wo2")]

        # ---- layer 0 ---- (mergeState x + seed*theta folded into xT host-side)
        xT_sb = []
        for fb in range(F // 128):
            t = b.state.tile([128, N], BF16, tag=f"hT{fb}", name="xTsb")
            nc.sync.dma_start(t[:, :], xT[fb * 128 : (fb + 1) * 128, :])
            xT_sb.append(t)

        # resident fp8 masks (full adjT, all layers; per-core output slice;
        # psel one-hot).  Issued on the otherwise-idle Pool queue so the
        # sync queue (f1 broadcasts etc.) is never head-of-line blocked
        # behind this ~11MB stream.
        mres = []
        for jb in range(NJB):
            t = b.mres.tile([128, N], FP8, tag=f"mres{jb}", name="mres")
            nc.gpsimd.dma_start(t[:, :], adjT[jb * 128 : (jb + 1) * 128, :])
            mres.append(t)

        vc0, af20 = b.fcols_tiles("l0")
        b.f2_cols_into(vc0, af20, xT_sb, wa02_sb, 0, NJB // 2)
        b.f2_cols_into(vc0, af20, xT_sb, wa02_sb, NJB // 2, NJB)
        f1row = b.state.tile([1, N], BF16, tag="f1row")
        f1b0 = b.state.tile([128, N], BF16, tag="f1bc0")
        b.f1_part(xT_sb, wa01_sb, f1row, f1b0, "f1bc0", 0, N,
                  pre_attention=True)
        whnm0 = b.project(xT_sb, w0_sb, D,
                          pools=[(b.ps_sm, "sm"), (b.ps_agg, "agg0"),
                                 (b.ps_agg, "agg1"), (b.ps_agg, "agg2")],
                          copy_dve=True)
        h0T = b.state.tile([D, N], BF16, tag="h0T")
        l1_pre = {}
        vc1, af21 = b.fcols_tiles("l1")
        f1b1 = b.state.tile([128, N], BF16, tag="f1bc1")

        def emit_l1_early():
            # Safe early emission: everything here reads only h0T columns
            # < HALF, whose writes are already emitted (Tile orders an
            # earlier-emitted read BEFORE a later-emitted write).  Separate
            # tags (whnmB, f1bc1) so these writes don't WAR-stall on
            # layer-0's still-live tiles.
            l1_pre["whnm_a"] = b.project([h0T], w1_sb, D,
                                         jb_range=(0, NJB // 2), tagp="whnmB")
            b.f2_cols_into(vc1, af21, [h0T], wa12_sb, 0, NJB // 2)
            b.f1_part([h0T], wa11_sb, f1row, f1b1, "f1bc1", 0, HALF)

        b.attention_agg(mres, 0, vc0, af20, f1b0, whnm0, D, N,
                        h0T, out_elu=True, between=emit_l1_early)

        # output-layer resident tiles: emitted only now (consumed post-
        # collective) so their DMA traffic stays off the startup window
        mosl = []
        for jb in range(NJB):
            t = b.mres.tile([128, ISL], FP8, tag=f"mosl{jb}", name="mosl")
            nc.gpsimd.dma_start(t[:, :],
                                adjT_osl[jb * 128 : (jb + 1) * 128, :])
            mosl.append(t)
        psel_sb = []
        for p0 in range(NJB // 2):
            t = b.mres.tile([128, 2 * ISL], FP8, tag=f"psel{p0}", name="psel")
            nc.gpsimd.dma_start(t[:, :], psel[p0 * 128 : (p0 + 1) * 128, :])
            psel_sb.append(t)

        # ---- layer 1 ---- (first halves of whnm/f2/f1 emitted early)
        whnm1 = l1_pre["whnm_a"] + b.project([h0T], w1_sb, D,
                                             jb_range=(NJB // 2, NJB),
                                             tagp="whnmB")
        b.f2_cols_into(vc1, af21, [h0T], wa12_sb, NJB // 2, NJB)
        b.f1_part([h0T], wa11_sb, f1row, f1b1, "f1bc1", HALF, N)
        h1T = b.state.tile([D, N], BF16, tag="h1T")

        def _ar_ap(offset, ap):
            base = ar_in[0:1, 0:1]
            return bass.AP(base.tensor, offset, ap)

        def _who_pre(jlo, jhi):
            # Everything the out layer needs is LINEAR in Who, so each
            # core stages its per-head contribution and the AllReduce sums
            # it: who_nm[jb] = (elu(h1) @ Wo) node-major [128, OUT] and the
            # f1/f2 logit columns (host-folded wo1 = Wo@ao1, wo2 = Wo@ao2).
            nb0 = jhi - jlo
            big = b.misc.tile([128, nb0 * OUT], BF16, tag="wnm_big")
            for u, jb in enumerate(range(jlo, jhi)):
                ps = b.ps_sm.tile([128, OUT], FP32, tag="sm", name="sm")
                nc.tensor.matmul(
                    ps[:, :], h1T[:, jb * 128 : (jb + 1) * 128],
                    wo_sb[0][:, :], start=True, stop=True,
                )
                nc.scalar.activation(big[:, u * OUT : (u + 1) * OUT],
                                     ps[:, :], AF.Copy)
            nc.sync.dma_start(
                _ar_ap(REG_NM + jlo * 128 * OUT,
                       [[OUT, 128], [128 * OUT, nb0], [1, OUT]]),
                big[:, :])
            nb = jhi - jlo
            fc = b.ps_sm.tile([128, 2 * nb], FP32, tag="sm", name="sm")
            for u, jb in enumerate(range(jlo, jhi)):
                nc.tensor.matmul(
                    fc[:, u : u + 1],
                    h1T[:, jb * 128 : (jb + 1) * 128], wo2_sb[0][:, :],
                    start=True, stop=True,
                )
                nc.tensor.matmul(
                    fc[:, nb + u : nb + u + 1],
                    h1T[:, jb * 128 : (jb + 1) * 128], wo1_sb[0][:, :],
                    start=True, stop=True,
                )
            t = b.misc.tile([128, 2 * nb], BF16, tag="fc_tmp")
            nc.scalar.activation(t[:, :], fc[:, :], AF.Copy)
            nc.sync.dma_start(
                _ar_ap(REG_F2 + jlo, [[NJB, 128], [1, nb]]), t[:, 0:nb])
            nc.sync.dma_start(
                _ar_ap(REG_F1 + jlo, [[NJB, 128], [1, nb]]),
                t[:, nb : 2 * nb])

        def emit_who_early():
            # h1T cols < HALF are fully emitted after half 0; stage the
            # first half of the AllReduce input early.
            _who_pre(0, NJB // 2)

        b.attention_agg(mres, 0, vc1, af21, f1b1, whnm1, D, N,
                        h1T, out_elu=True, between=emit_who_early)

        if dbg:
            nc.sync.dma_start(h0_dbg[:, :], h0T[:, :])
            nc.sync.dma_start(h1_dbg[:, :], h1T[:, :])

        # ---- out layer via AllReduce of per-head contributions ----
        _who_pre(NJB // 2, NJB)  # second half (first staged early)
        if sim or os.environ.get("GAT_NO_COLL"):
            nc.sync.dma_start(ar_out[:, :], ar_in[:, :])
        else:
            nc.gpsimd.collective_compute(
                "AllReduce", ALU.add,
                replica_groups=[list(range(NCORES))],
                ins=[ar_in.ap().opt()], outs=[ar_out.ap().opt()],
            )

        def _ar_out_ap(offset, ap):
            base = ar_out[0:1, 0:1]
            return bass.AP(base.tensor, offset, ap)

        whnmo_all = b.state.tile([128, NJB * OUT], BF16, tag="whnmo")
        nc.sync.dma_start(
            whnmo_all[:, :],
            _ar_out_ap(REG_NM, [[OUT, 128], [128 * OUT, NJB], [1, OUT]]))
        whnmo = [whnmo_all[:, jb * OUT : (jb + 1) * OUT] for jb in range(NJB)]
        fcols = b.state.tile([128, 2 * NJB], BF16, tag="fcols")
        nc.sync.dma_start(fcols[:, :],
                          _ar_out_ap(REG_F2, [[NJB, 128], [N, 2], [1, NJB]]))
        f2colb = fcols[:, 0:NJB]
        f1colb = fcols[:, NJB : 2 * NJB]
        vco = b.state.tile([128, NJB], FP32, tag="lov")
        nc.scalar.activation(vco[:, :], f2colb[:, :], AF.Exp)
        af2o = b.state.tile([128, NJB], FP32, tag="loa")
        nc.scalar.activation(af2o[:, :], f2colb[:, :], AF.Copy, scale=ALPHA)
        # f1col as fp8 at column-stride 16 so pairs form DoubleRow lhsT
        # [128, 2, 1] APs (error is per-column in the softmax -> cancels)
        f1c8 = b.misc.tile([128, 16 * NJB], FP8, tag="f1c8")
        f1c8_ap = bass.AP(f1c8[:, :].tensor, f1c8[:, :].offset,
                          [list(f1c8[:, :].ap[0])] + [[16, NJB]])
        nc.scalar.activation(f1c8_ap, f1colb[:, :], AF.Copy)
        f1sps = b.ps_sm.tile([1, 512], FP32, tag="sm", name="sm")
        for p0 in range(NJB // 2):
            base = f1c8[:, 32 * p0 : 32 * p0 + 17]
            lhsT = bass.AP(base.tensor, base.offset,
                           [list(base.ap[0])] + [[16, 2], [1, 1]])
            pbase = psel_sb[p0][:, :]
            rhs = bass.AP(pbase.tensor, pbase.offset,
                          [list(pbase.ap[0])] + [[ISL, 2], [1, ISL]])
            nc.tensor.matmul(
                f1sps[:, :ISL], lhsT, rhs,
                start=(p0 == 0), stop=(p0 == NJB // 2 - 1),
                perf_mode=mybir.MatmulPerfMode.DoubleRow,
            )
        f1slrow = b.state.tile([1, ISL], BF16, tag="f1slrow")
        nc.scalar.activation(f1slrow[:, :], f1sps[:, :ISL], AF.Copy)
        # broadcast via a ones-matmul into a free PSUM bank (the out-layer
        # attention only uses agg0/rs0) -- saves the serial DRAM roundtrip
        f1bo_ps = b.ps_agg.tile([128, ISL], FP32, tag="agg2", name="agg2")
        nc.tensor.matmul(f1bo_ps[:, :], b.ones1b[:, :128], f1slrow[:, :],
                         start=True, stop=True)
        f1bo = b.misc.tile([128, ISL], BF16, tag="f1bo")
        nc.scalar.activation(f1bo[:, :], f1bo_ps[:, :], AF.Copy)
        o_fin = b.state.tile([OUT, ISL], FP32, tag="o_fin")
        b.attention_agg(mosl, 0, vco, af2o, f1bo, whnmo, OUT, ISL,
                        o_fin, out_elu=False)
        # final ELU
        m = b.misc.tile([OUT, ISL], FP32, tag="fin_m")
        nc.vector.tensor_scalar(m[:, :], o_fin[:, :], 0.0, None, ALU.min)
        e = b.misc.tile([OUT, ISL], FP32, tag="fin_e")
        nc.scalar.activation(e[:, :], m[:, :], AF.Exp)
        r = b.misc.tile([OUT, ISL], FP32, tag="fin_r")
        nc.vector.tensor_scalar(r[:, :], o_fin[:, :], 0.0, -1.0, ALU.max,
                                ALU.add)
        fin = b.misc.tile([OUT, ISL], FP32, tag="fin")
        nc.vector.tensor_add(fin[:, :], e[:, :], r[:, :])
        nc.sync.dma_start(outT[:, :], fin[:, :])
    nc.compile()
    return nc


def make_in_maps(inputs):
    x = np.asarray(inputs["x"], np.float32)
    adj = np.asarray(inputs["adj"], np.float32)
    observation = np.asarray(inputs["observation"])
    theta = np.asarray(inputs["theta"], np.float32)
    W0 = np.asarray(inputs["W0"], np.float32)
    a0 = np.asarray(inputs["a0"], np.float32)
    W1 = np.asarray(inputs["W1"], np.float32)
    a1 = np.asarray(inputs["a1"], np.float32)
    Wo = np.asarray(inputs["Wo"], np.float32)
    ao = np.asarray(inputs["ao"], np.float32)

    bf = ml_dtypes.bfloat16
    f8 = ml_dtypes.float8_e4m3
    seed = (observation[0] == 1).astype(np.float32)[:, None]
    h = x + seed * theta                     # mergeState folded host-side
    hT = np.ascontiguousarray(h.T).astype(bf)
    adjT = np.ascontiguousarray((adj > 0).T.astype(np.float32)).astype(f8)
    wo_bf = Wo.astype(bf)

    in_maps = []
    for c in range(NCORES):
        psel_f = np.zeros((N, ISL), np.float32)
        psel_f[c * ISL + np.arange(ISL), np.arange(ISL)] = 1.0
        psel_c = np.concatenate(
            [np.concatenate([psel_f[2 * p * 128 : (2 * p + 1) * 128],
                             psel_f[(2 * p + 1) * 128 : (2 * p + 2) * 128]],
                            axis=1) for p in range(NJB // 2)], axis=0)
        in_maps.append({
            "psel": psel_c.astype(f8),
            "xT": hT,
            "adjT": adjT,
            "adjT_osl": np.ascontiguousarray(adjT[:, c * ISL : (c + 1) * ISL]),
            "w0": W0[c].astype(bf),
            "a01": (W0[c] @ a0[c][:D]).astype(bf),
            "a02": (W0[c] @ a0[c][D:]).astype(bf),
            "w1": W1[c].astype(bf),
            "a11": (W1[c] @ a1[c][:D]).astype(bf),
            "a12": (W1[c] @ a1[c][D:]).astype(bf),
            "wo": np.ascontiguousarray(wo_bf[c * D : (c + 1) * D]),
            "wo1": (Wo[c * D : (c + 1) * D] @ ao[:OUT]).astype(bf),
            "wo2": (Wo[c * D : (c + 1) * D] @ ao[OUT:]).astype(bf),
        })
    return in_maps


def kernel(**inputs):
    in_maps = make_in_maps(inputs)
    nc = build()
    res = run_bass_kernel_spmd(nc, in_maps, core_ids=list(range(NCORES)))
    out = np.concatenate(
        [res.results[c]["outT"].T for c in range(NCORES)], axis=0
    )
    return np.ascontiguousarray(out, np.float32)


if __name__ == "__main__":
    build()
    print("built ok")


# revision 59
# speedup vs baseline: 1.2438x; 1.2438x over previous
"""Trainium2 Bass kernel for nn_GAT_38989713113447 (3-layer dense GAT).

Sharding: 8 heads over 8 cores for the two inner GAT layers (pure head
parallelism, no communication).  The head-concat + output projection
commutes into a sum of per-head projections: Who = sum_k h1_k @ Wo[k],
and EVERYTHING the output layer consumes is linear in Who, so each core
pre-stages its head's contribution to (a) node-major Who blocks
[24][128, 64] and (b) the f1/f2 logit columns (via host-folded
wo1 = Wo@ao1, wo2 = Wo@ao2) into one packed [66, 3072] buffer that a
single AllReduce(add) sums.  Post-collective work is just two coalesced
DMA loads + exp/scale + the masked output attention.  The output
attention is sharded over node rows (384 rows/core); the per-core
column slice of f1 is selected via a one-hot matmul against a per-core
psel input (keeps the SPMD program identical on every core).  The
final [3072, 64] output is assembled host-side from per-core slices.

Attention math (per column i, softmax over j):  the per-column factor
exp(f1_i) cancels in softmax, so with
    at[j,i]  = m * max(exp(f1_i+f2_j), exp(a*f1_i + a*f2_j))      (a=0.2)
    at'[j,i] = at[j,i] / exp(f1_i)
             = m * max(exp(f2_j), exp((a-1)*f1_i + a*f2_j))
the first branch is per-PARTITION constant v_j = exp(f2_j).  One ACT
pass builds e1' = Exp((a-1)*f1bc + a*f2_j) (bias per-partition), one
DVE scalar_tensor_tensor builds at' = (e1' max v_j) * m.  That is 1 ACT
+ 1 DVE pass over [N,N] per layer (the unfactored form needs 2 DVE).

mergeState (x + seed*theta) is folded into the hT input host-side.

dtypes: masks and at' are fp8e4 (values O(1), exact {0,1} masks); the
adjacency mask is DMA'd once (on the idle Pool queue) and stays
SBUF-resident across all three layers.  Aggregation matmuls run mixed
lhsT=bf16(Wh) x rhs=fp8(at'); softmax denominators use fp8 DoubleRow
matmuls (2 j-tiles per pass) against an interleaved at-pair layout.
"""

import os
import sys

sys.path.insert(0, "/opt/trn_rl_repo")

from contextlib import ExitStack

import numpy as np
import ml_dtypes

import concourse.bass as bass  # noqa: F401
import concourse.bacc as bacc
import concourse.tile as tile
from concourse import mybir
from concourse.bass_utils import run_bass_kernel_spmd

N = 3072
F = 256
H = 8
D = 128          # H1 == H2
OUT = 64
ALPHA = 0.2
NCORES = 8
NJB = N // 128   # 24 attention j-blocks
HALF = N // 2    # i-dim half per PSUM residency
ISL = N // NCORES  # 384 output rows per core

FP32 = mybir.dt.float32
BF16 = mybir.dt.bfloat16
FP8 = mybir.dt.float8e4
AF = mybir.ActivationFunctionType
ALU = mybir.AluOpType

def _chunks(total, step):
    return [(o, min(step, total - o)) for o in range(0, total, step)]


class Builder:
    def __init__(self, nc, tc, ctx):
        self.nc = nc
        self.tc = tc
        p = lambda name, bufs, space=None: ctx.enter_context(
            tc.tile_pool(name=name, bufs=bufs, **({"space": space} if space else {}))
        )
        self.state = p("state", 1)
        self.mres = p("mres", 1)
        self.work = p("work", 4)
        self.att = p("att", 4)
        self.ps_agg = p("ps_agg", 1, "PSUM")
        self.ps_rs = p("ps_rs", 1, "PSUM")
        self.ps_sm = p("ps_sm", 2, "PSUM")
        self.misc = p("misc", 1)

    def ones_tile(self, shape, dtype, name):
        t = self.state.tile(shape, dtype, tag=name, name=name)
        self.nc.vector.memset(t[:, :], 1.0)
        return t

    def ones_pair(self):
        """fp8 [128, 2, 1] stationary AP for DoubleRow rowsum (cols 0, 16)."""
        t = self.state.tile([128, 32], FP8, tag="ones_pair", name="ones_pair")
        nc = self.nc
        nc.vector.memset(t[:, :], 0.0)
        nc.vector.memset(t[:, 0:1], 1.0)
        nc.vector.memset(t[:, 16:17], 1.0)
        ap = t[:, :]
        return bass.AP(ap.tensor, ap.offset,
                       [list(ap.ap[0])] + [[16, 2], [1, 1]])

    def _rowd(self, tag, rows=1):
        if not hasattr(self, "_row_dram"):
            self._row_dram = {}
        if tag not in self._row_dram:
            self._row_dram[tag] = self.nc.dram_tensor(
                f"rowd_{tag}", [rows, N], BF16)
        return self._row_dram[tag]

    def bcast_row(self, row_ap, width, tag):
        """[1, width] bf16 SBUF row -> [128, width] bf16 tile via a DMA with
        a partition-step-0 source AP (reads the row 128x).  Issued on the
        Activation HWDGE queue: these sit at layer boundaries and would
        head-of-line block the sync or Pool queues."""
        nc = self.nc
        rd = self._rowd(tag)
        out = self.state.tile([128, width], BF16, tag=tag)
        for off, w in _chunks(width, 512):
            nc.scalar.dma_start(rd[:, off : off + w],
                                row_ap[:, off : off + w])
            rs_ap = rd[:1, off : off + w]
            bsrc = bass.AP(rs_ap.tensor, rs_ap.offset,
                           [[0, 128]] + [list(p) for p in rs_ap.ap])
            nc.scalar.dma_start(out[:, off : off + w], bsrc)
        return out

    def project(self, hT_tiles, w_sb, dT, jb_range=None, tagp="whnm",
                pools=None, copy_dve=False):
        """Node-major Wh_nm[jb][j, dT] = (h @ W) blocks.  pools: optional
        list of (pool, tag) to rotate PSUM tiles through (the attention
        pools are free outside the attention loops -> deeper buffering).
        copy_dve: do PSUM->SBUF copies on DVE (idle at startup) so the ACT
        queue stays clear for the f1row/f1bc chain."""
        nc = self.nc
        nk = len(hT_tiles)
        wh_nm = []
        per = max(1, 512 // dT)  # jb blocks packed per PSUM tile
        jlo, jhi = jb_range if jb_range is not None else (0, NJB)
        for gi, jb0 in enumerate(range(jlo, jhi, per)):
            njb = min(per, jhi - jb0)
            if pools:
                pool, ptag = pools[gi % len(pools)]
                ps = pool.tile([128, 512], FP32, tag=ptag, name=ptag)
            else:
                ps = self.ps_sm.tile([128, 512], FP32, tag="sm", name="sm")
            for u in range(njb):
                jb = jb0 + u
                for k in range(nk):
                    nc.tensor.matmul(
                        ps[:, u * dT : (u + 1) * dT],
                        hT_tiles[k][:, jb * 128 : (jb + 1) * 128],
                        w_sb[k][:, :],
                        start=(k == 0), stop=(k == nk - 1),
                    )
            t = self.state.tile([128, per * dT], BF16,
                                tag=f"{tagp}_{jb0 // per}", name="whnm")
            if copy_dve:
                nc.vector.tensor_copy(t[:, : njb * dT], ps[:, : njb * dT])
            else:
                nc.scalar.activation(t[:, : njb * dT], ps[:, : njb * dT],
                                     AF.Copy)
            for u in range(njb):
                wh_nm.append(t[:, u * dT : (u + 1) * dT])
        return wh_nm

    def f2_cols_into(self, vcol, af2col, hT_tiles, wa2_sb, jlo, jhi):
        """Columns [jlo, jhi) of vcol = exp(f2col), af2col = ALPHA*f2col.
        f2[jb] depends only on h columns of block jb, so the first half is
        emittable mid-attention (between i-halves)."""
        nc = self.nc
        nk = len(hT_tiles)
        nb = jhi - jlo
        f2ps = self.ps_sm.tile([128, NJB], FP32, tag="sm", name="sm")
        for u, jb in enumerate(range(jlo, jhi)):
            for k in range(nk):
                nc.tensor.matmul(
                    f2ps[:, u : u + 1],
                    hT_tiles[k][:, jb * 128 : (jb + 1) * 128],
                    wa2_sb[k][:, :],
                    start=(k == 0), stop=(k == nk - 1),
                )
        nc.scalar.activation(vcol[:, jlo:jhi], f2ps[:, :nb], AF.Exp)
        nc.scalar.activation(af2col[:, jlo:jhi], f2ps[:, :nb], AF.Copy,
                             scale=ALPHA)

    def fcols_tiles(self, tag):
        vcol = self.state.tile([128, NJB], FP32, tag=tag + "v")
        af2col = self.state.tile([128, NJB], FP32, tag=tag + "a")
        return vcol, af2col

    def f1_part(self, hT_tiles, wa1_sb, f1row, f1bc, rowd_tag, lo, hi,
                pre_attention=False):
        """f1row/f1bc columns [lo, hi): row matmuls + broadcast DMAs."""
        nc = self.nc
        nk = len(hT_tiles)
        rd = self._rowd(rowd_tag)
        for off, w in _chunks(hi - lo, 512):
            o = lo + off
            ps = self.ps_sm.tile([1, 512], FP32, tag="sm", name="sm")
            for k in range(nk):
                nc.tensor.matmul(
                    ps[:, :w], wa1_sb[k][:, :],
                    hT_tiles[k][:, o : o + w],
                    start=(k == 0), stop=(k == nk - 1),
                )
            nc.scalar.activation(f1row[:, o : o + w], ps[:, :w], AF.Copy)
            if pre_attention:
                # the rowsum PSUM banks are free pre-attention: broadcast
                # via a PE ones-matmul + DVE copy (no DMA latency/contention)
                bp = self.ps_rs.tile([128, 512], FP32,
                                     tag=f"rs{(o // 512) % 3}", name="rs")
                nc.tensor.matmul(bp[:, :w], self.ones1b[:, :128],
                                 f1row[:, o : o + w], start=True, stop=True)
                nc.vector.tensor_copy(f1bc[:, o : o + w], bp[:, :w])
            else:
                # sync queue: mid-attention the ACT queue is saturated with
                # e1 ops and a waiting DMA would head-of-line block them
                nc.sync.dma_start(rd[:, o : o + w], f1row[:, o : o + w])
        if not pre_attention:
            # reads in a second pass so the write->read pairs pipeline
            for off, w in _chunks(hi - lo, 512):
                o = lo + off
                rs_ap = rd[:1, o : o + w]
                bsrc = bass.AP(rs_ap.tensor, rs_ap.offset,
                               [[0, 128]] + [list(p) for p in rs_ap.ap])
                nc.sync.dma_start(f1bc[:, o : o + w], bsrc)

    def attention_agg(self, mres, mcol, vcol, af2col, f1bc, wh_nm, dT,
                      width, h_out, out_elu, between=None):
        """Factored masked softmax + aggregation + normalize (+ELU).

        mres: list of NJB resident fp8 mask tiles; mcol: column offset into
        them.  between: callback after the first i-half's emission (its
        inputs must be fully emitted by then)."""
        nc = self.nc
        half_w = min(width, HALF)
        for h0 in range(0, width, half_w):
            if h0 > 0 and between is not None:
                between()
                between = None
            hw = min(half_w, width - h0)
            ch = _chunks(hw, 512)
            agg_ps = [self.ps_agg.tile([dT, 512], FP32, tag=f"agg{ci}",
                                       name=f"agg{ci}")
                      for ci in range(len(ch))]
            rs_ps = [self.ps_rs.tile([1, 512], FP32, tag=f"rs{ci}",
                                     name=f"rs{ci}")
                     for ci in range(len(ch))]
            npair = NJB // 2
            n_gp = int(os.environ.get("GAT_GP_STT", "0"))
            for pair in range(npair):
                atp = self.att.tile([128, 2 * hw], FP8, tag="atp")
                on_gp = n_gp and (pair % (npair // max(n_gp, 1)) ==
                                  npair // max(n_gp, 1) - 1)
                eng = nc.gpsimd if on_gp else nc.vector
                for side in range(2):
                    jb = 2 * pair + side
                    e1 = self.work.tile([128, hw], FP8, tag="e1")
                    nc.scalar.activation(
                        e1[:, :], f1bc[:, h0 : h0 + hw], AF.Exp,
                        scale=ALPHA - 1.0, bias=af2col[:, jb : jb + 1],
                    )
                    eng.scalar_tensor_tensor(
                        atp[:, side * hw : (side + 1) * hw], e1[:, :],
                        vcol[:, jb : jb + 1],
                        mres[jb][:, mcol + h0 : mcol + h0 + hw],
                        ALU.max, ALU.mult,
                    )
                for side in range(2):
                    jb = 2 * pair + side
                    for ci, (off, w) in enumerate(ch):
                        nc.tensor.matmul(
                            agg_ps[ci][:, :w], wh_nm[jb],
                            atp[:, side * hw + off : side * hw + off + w],
                            start=(jb == 0), stop=(jb == NJB - 1),
                        )
                for ci, (off, w) in enumerate(ch):
                    base = atp[:, off : off + w]
                    rhs = bass.AP(base.tensor, base.offset,
                                  [list(base.ap[0])] + [[hw, 2], [1, w]])
                    nc.tensor.matmul(
                        rs_ps[ci][:, :w], self.ones_pair_ap, rhs,
                        start=(pair == 0), stop=(pair == npair - 1),
                        perf_mode=mybir.MatmulPerfMode.DoubleRow,
                    )
            # softmax denominator -> reciprocal -> broadcast -> normalize
            rinv = self.misc.tile([1, hw], FP32, tag="rinv")
            for ci, (off, w) in enumerate(ch):
                nc.vector.reciprocal_approx_fast(rinv[:, off : off + w],
                                                 rs_ps[ci][:, :w])
            rb_sb = self.misc.tile([dT, hw], BF16, tag="rb_sb")
            for off, w in _chunks(hw, 512):
                ps = self.ps_sm.tile([dT, 512], FP32, tag="sm", name="sm")
                nc.tensor.matmul(
                    ps[:, :w], self.ones1f[:, :dT], rinv[:, off : off + w],
                    start=True, stop=True,
                )
                nc.scalar.activation(rb_sb[:, off : off + w], ps[:, :w], AF.Copy)
            hpn = self.misc.tile([dT, hw], FP32 if not out_elu else BF16,
                                 tag="hpn")
            # stage agg out of PSUM via ACT (has slack) so the normalize
            # multiply runs on DVE (the binding engine) at 2x_1p bf16
            # instead of 1x with the PSUM access penalty
            asb = self.misc.tile([dT, hw], BF16, tag="asb")
            for ci, (off, w) in enumerate(ch):
                if out_elu:
                    nc.scalar.activation(asb[:, off : off + w],
                                         agg_ps[ci][:dT, :w], AF.Copy)
                    nc.vector.tensor_tensor(
                        hpn[:, off : off + w], asb[:, off : off + w],
                        rb_sb[:, off : off + w], ALU.mult,
                    )
                else:
                    nc.vector.tensor_tensor(
                        hpn[:, off : off + w], agg_ps[ci][:dT, :w],
                        rb_sb[:, off : off + w], ALU.mult,
                    )
            if out_elu:
                # ELU(x) = exp(min(x,0)) - 1 + max(x,0)
                m = self.misc.tile([dT, hw], BF16, tag="elu_m")
                nc.vector.tensor_scalar(m[:, :], hpn[:, :], 0.0, None, ALU.min)
                e = self.misc.tile([dT, hw], BF16, tag="elu_e")
                nc.scalar.activation(e[:, :], m[:, :], AF.Exp)
                r = self.misc.tile([dT, hw], BF16, tag="elu_m")
                nc.vector.tensor_scalar(
                    r[:, :], hpn[:, :], 0.0, -1.0, ALU.max, ALU.add
                )
                nc.vector.tensor_add(h_out[:, h0 : h0 + hw], e[:, :], r[:, :])
            else:
                nc.vector.tensor_copy(h_out[:, h0 : h0 + hw], hpn[:, :])


def build(dbg=False, sim=False):
    nc = bacc.Bacc("TRN2", target_bir_lowering=False, num_devices=NCORES)

    xT = nc.dram_tensor("xT", [F, N], BF16, kind="ExternalInput")
    adjT = nc.dram_tensor("adjT", [N, N], FP8, kind="ExternalInput")
    adjT_osl = nc.dram_tensor("adjT_osl", [N, ISL], FP8, kind="ExternalInput")
    w0 = nc.dram_tensor("w0", [F, D], BF16, kind="ExternalInput")
    a01 = nc.dram_tensor("a01", [F, 1], BF16, kind="ExternalInput")
    a02 = nc.dram_tensor("a02", [F, 1], BF16, kind="ExternalInput")
    w1 = nc.dram_tensor("w1", [D, D], BF16, kind="ExternalInput")
    a11 = nc.dram_tensor("a11", [D, 1], BF16, kind="ExternalInput")
    a12 = nc.dram_tensor("a12", [D, 1], BF16, kind="ExternalInput")
    wo = nc.dram_tensor("wo", [D, OUT], BF16, kind="ExternalInput")
    wo1 = nc.dram_tensor("wo1", [D, 1], BF16, kind="ExternalInput")
    wo2 = nc.dram_tensor("wo2", [D, 1], BF16, kind="ExternalInput")
    psel = nc.dram_tensor("psel", [NJB // 2 * 128, 2 * ISL], FP8,
                          kind="ExternalInput")

    outT = nc.dram_tensor("outT", [OUT, ISL], FP32, kind="ExternalOutput")
    if dbg:
        h0_dbg = nc.dram_tensor("h0_dbg", [D, N], BF16, kind="ExternalOutput")
        h1_dbg = nc.dram_tensor("h1_dbg", [D, N], BF16, kind="ExternalOutput")

    ar_in = nc.dram_tensor("ar_in", [OUT + 2, N], BF16)
    ar_out = nc.dram_tensor("ar_out", [OUT + 2, N], BF16, addr_space="Shared")
    REG_NM, REG_F2, REG_F1 = 0, OUT * N, OUT * N + N

    with tile.TileContext(nc) as tc, ExitStack() as ctx:
        b = Builder(nc, tc, ctx)
        b.ones1f = b.ones_tile([1, 128], FP32, "ones1f")
        b.ones1b = b.ones_tile([1, 128], BF16, "ones1b")
        b.ones_pair_ap = b.ones_pair()

        def load_w(ap, shape, tag, dt=BF16):
            s = b.state.tile(shape, dt, tag=tag, name=tag)
            nc.sync.dma_start(s[:, :], ap)
            return s

        w0_sb = [load_w(w0[k * 128 : (k + 1) * 128, :], [128, D], f"w0_{k}")
                 for k in range(F // 128)]
        wa01_sb = [load_w(a01[k * 128 : (k + 1) * 128, :], [128, 1],
                          f"wa01_{k}") for k in range(F // 128)]
        wa02_sb = [load_w(a02[k * 128 : (k + 1) * 128, :], [128, 1],
                          f"wa02_{k}") for k in range(F // 128)]
        w1_sb = [load_w(w1[:, :], [D, D], "w1")]
        wa11_sb = [load_w(a11[:, :], [D, 1], "wa11")]
        wa12_sb = [load_w(a12[:, :], [D, 1], "wa12")]
        wo_sb = [load_w(wo[:, :], [D, OUT], "wo")]
        wo1_sb = [load_w(wo1[:, :], [D, 1], "wo1")]
        wo2_sb = [load_w(wo2[:, :], [D, 1], "wo2")]

        # ---- layer 0 ---- (mergeState x + seed*theta folded into xT host-side)
        xT_sb = []
        for fb in range(F // 128):
            t = b.state.tile([128, N], BF16, tag=f"hT{fb}", name="xTsb")
            nc.sync.dma_start(t[:, :], xT[fb * 128 : (fb + 1) * 128, :])
            xT_sb.append(t)

        # resident fp8 masks (full adjT, all layers; per-core output slice;
        # psel one-hot).  Issued on the otherwise-idle Pool queue so the
        # sync queue (f1 broadcasts etc.) is never head-of-line blocked
        # behind this ~11MB stream.
        mres = []
        for jb in range(NJB):
            t = b.mres.tile([128, N], FP8, tag=f"mres{jb}", name="mres")
            nc.gpsimd.dma_start(t[:, :], adjT[jb * 128 : (jb + 1) * 128, :])
            mres.append(t)

        vc0, af20 = b.fcols_tiles("l0")
        b.f2_cols_into(vc0, af20, xT_sb, wa02_sb, 0, NJB // 2)
        b.f2_cols_into(vc0, af20, xT_sb, wa02_sb, NJB // 2, NJB)
        f1row = b.state.tile([1, N], BF16, tag="f1row")
        f1b0 = b.state.tile([128, N], BF16, tag="f1bc0")
        b.f1_part(xT_sb, wa01_sb, f1row, f1b0, "f1bc0", 0, N,
                  pre_attention=True)
        whnm0 = b.project(xT_sb, w0_sb, D,
                          pools=[(b.ps_sm, "sm"), (b.ps_agg, "agg0"),
                                 (b.ps_agg, "agg1"), (b.ps_agg, "agg2")],
                          copy_dve=True)
        h0T = b.state.tile([D, N], BF16, tag="h0T")
        l1_pre = {}
        vc1, af21 = b.fcols_tiles("l1")
        f1b1 = b.state.tile([128, N], BF16, tag="f1bc1")

        def emit_l1_early():
            # Safe early emission: everything here reads only h0T columns
            # < HALF, whose writes are already emitted (Tile orders an
            # earlier-emitted read BEFORE a later-emitted write).  Separate
            # tags (whnmB, f1bc1) so these writes don't WAR-stall on
            # layer-0's still-live tiles.
            l1_pre["whnm_a"] = b.project([h0T], w1_sb, D,
                                         jb_range=(0, NJB // 2), tagp="whnmB")
            b.f2_cols_into(vc1, af21, [h0T], wa12_sb, 0, NJB // 2)
            b.f1_part([h0T], wa11_sb, f1row, f1b1, "f1bc1", 0, HALF)

        b.attention_agg(mres, 0, vc0, af20, f1b0, whnm0, D, N,
                        h0T, out_elu=True, between=emit_l1_early)

        # output-layer resident tiles: emitted only now (consumed post-
        # collective) so their DMA traffic stays off the startup window
        mosl = []
        for jb in range(NJB):
            t = b.mres.tile([128, ISL], FP8, tag=f"mosl{jb}", name="mosl")
            nc.gpsimd.dma_start(t[:, :],
                                adjT_osl[jb * 128 : (jb + 1) * 128, :])
            mosl.append(t)
        psel_sb = []
        for p0 in range(NJB // 2):
            t = b.mres.tile([128, 2 * ISL], FP8, tag=f"psel{p0}", name="psel")
            nc.gpsimd.dma_start(t[:, :], psel[p0 * 128 : (p0 + 1) * 128, :])
            psel_sb.append(t)

        # ---- layer 1 ---- (first halves of whnm/f2/f1 emitted early)
        whnm1 = l1_pre["whnm_a"] + b.project([h0T], w1_sb, D,
                                             jb_range=(NJB // 2, NJB),
                                             tagp="whnmB")
        b.f2_cols_into(vc1, af21, [h0T], wa12_sb, NJB // 2, NJB)
        b.f1_part([h0T], wa11_sb, f1row, f1b1, "f1bc1", HALF, N)
        h1T = b.state.tile([D, N], BF16, tag="h1T")

        def _ar_ap(offset, ap):
            base = ar_in[0:1, 0:1]
            return bass.AP(base.tensor, offset, ap)

        def _who_pre(jlo, jhi):
            # Everything the out layer needs is LINEAR in Who, so each
            # core stages its per-head contribution and the AllReduce sums
            # it: who_nm[jb] = (elu(h1) @ Wo) node-major [128, OUT] and the
            # f1/f2 logit columns (host-folded wo1 = Wo@ao1, wo2 = Wo@ao2).
            nb0 = jhi - jlo
            big = b.misc.tile([128, nb0 * OUT], BF16, tag="wnm_big")
            for u, jb in enumerate(range(jlo, jhi)):
                ps = b.ps_sm.tile([128, OUT], FP32, tag="sm", name="sm")
                nc.tensor.matmul(
                    ps[:, :], h1T[:, jb * 128 : (jb + 1) * 128],
                    wo_sb[0][:, :], start=True, stop=True,
                )
                nc.scalar.activation(big[:, u * OUT : (u + 1) * OUT],
                                     ps[:, :], AF.Copy)
            nc.sync.dma_start(
                _ar_ap(REG_NM + jlo * 128 * OUT,
                       [[OUT, 128], [128 * OUT, nb0], [1, OUT]]),
                big[:, :])
            nb = jhi - jlo
            fc = b.ps_sm.tile([128, 2 * nb], FP32, tag="sm", name="sm")
            for u, jb in enumerate(range(jlo, jhi)):
                nc.tensor.matmul(
                    fc[:, u : u + 1],
                    h1T[:, jb * 128 : (jb + 1) * 128], wo2_sb[0][:, :],
                    start=True, stop=True,
                )
                nc.tensor.matmul(
                    fc[:, nb + u : nb + u + 1],
                    h1T[:, jb * 128 : (jb + 1) * 128], wo1_sb[0][:, :],
                    start=True, stop=True,
                )
            t = b.misc.tile([128, 2 * nb], BF16, tag="fc_tmp")
            nc.scalar.activation(t[:, :], fc[:, :], AF.Copy)
            nc.sync.dma_start(
                _ar_ap(REG_F2 + jlo, [[NJB, 128], [1, nb]]), t[:, 0:nb])
            nc.sync.dma_start(
                _ar_ap(REG_F1 + jlo, [[NJB, 128], [1, nb]]),
                t[:, nb : 2 * nb])

        def emit_who_early():
            # h1T cols < HALF are fully emitted after half 0; stage the
            # first half of the AllReduce input early.
            _who_pre(0, NJB // 2)

        b.attention_agg(mres, 0, vc1, af21, f1b1, whnm1, D, N,
                        h1T, out_elu=True, between=emit_who_early)

        if dbg:
            nc.sync.dma_start(h0_dbg[:, :], h0T[:, :])
            nc.sync.dma_start(h1_dbg[:, :], h1T[:, :])

        # ---- out layer via AllReduce of per-head contributions ----
        _who_pre(NJB // 2, NJB)  # second half (first staged early)
        if sim or os.environ.get("GAT_NO_COLL"):
            nc.sync.dma_start(ar_out[:, :], ar_in[:, :])
        else:
            nc.gpsimd.collective_compute(
                "AllReduce", ALU.add,
                replica_groups=[list(range(NCORES))],
                ins=[ar_in.ap().opt()], outs=[ar_out.ap().opt()],
            )

        def _ar_out_ap(offset, ap):
            base = ar_out[0:1, 0:1]
            return bass.AP(base.tensor, offset, ap)

        whnmo_all = b.state.tile([128, NJB * OUT], BF16, tag="whnmo")
        nc.sync.dma_start(
            whnmo_all[:, :],
            _ar_out_ap(REG_NM, [[OUT, 128], [128 * OUT, NJB], [1, OUT]]))
        whnmo = [whnmo_all[:, jb * OUT : (jb + 1) * OUT] for jb in range(NJB)]
        fcols = b.state.tile([128, 2 * NJB], BF16, tag="fcols")
        nc.sync.dma_start(fcols[:, :],
                          _ar_out_ap(REG_F2, [[NJB, 128], [N, 2], [1, NJB]]))
        f2colb = fcols[:, 0:NJB]
        f1colb = fcols[:, NJB : 2 * NJB]
        vco = b.state.tile([128, NJB], FP32, tag="lov")
        nc.scalar.activation(vco[:, :], f2colb[:, :], AF.Exp)
        af2o = b.state.tile([128, NJB], FP32, tag="loa")
        nc.scalar.activation(af2o[:, :], f2colb[:, :], AF.Copy, scale=ALPHA)
        # f1col as fp8 at column-stride 16 so pairs form DoubleRow lhsT
        # [128, 2, 1] APs (error is per-column in the softmax -> cancels)
        f1c8 = b.misc.tile([128, 16 * NJB], FP8, tag="f1c8")
        f1c8_ap = bass.AP(f1c8[:, :].tensor, f1c8[:, :].offset,
                          [list(f1c8[:, :].ap[0])] + [[16, NJB]])
        nc.scalar.activation(f1c8_ap, f1colb[:, :], AF.Copy)
        f1sps = b.ps_sm.tile([1, 512], FP32, tag="sm", name="sm")
        for p0 in range(NJB // 2):
            base = f1c8[:, 32 * p0 : 32 * p0 + 17]
            lhsT = bass.AP(base.tensor, base.offset,
                           [list(base.ap[0])] + [[16, 2], [1, 1]])
            pbase = psel_sb[p0][:, :]
            rhs = bass.AP(pbase.tensor, pbase.offset,
                          [list(pbase.ap[0])] + [[ISL, 2], [1, ISL]])
            nc.tensor.matmul(
                f1sps[:, :ISL], lhsT, rhs,
                start=(p0 == 0), stop=(p0 == NJB // 2 - 1),
                perf_mode=mybir.MatmulPerfMode.DoubleRow,
            )
        f1slrow = b.state.tile([1, ISL], BF16, tag="f1slrow")
        nc.scalar.activation(f1slrow[:, :], f1sps[:, :ISL], AF.Copy)
        # broadcast via a ones-matmul into a free PSUM bank (the out-layer
        # attention only uses agg0/rs0) -- saves the serial DRAM roundtrip
        f1bo_ps = b.ps_agg.tile([128, ISL], FP32, tag="agg2", name="agg2")
        nc.tensor.matmul(f1bo_ps[:, :], b.ones1b[:, :128], f1slrow[:, :],
                         start=True, stop=True)
        f1bo = b.misc.tile([128, ISL], BF16, tag="f1bo")
        nc.scalar.activation(f1bo[:, :], f1bo_ps[:, :], AF.Copy)
        o_fin = b.state.tile([OUT, ISL], FP32, tag="o_fin")
        b.attention_agg(mosl, 0, vco, af2o, f1bo, whnmo, OUT, ISL,
                        o_fin, out_elu=False)
        # final ELU
        m = b.misc.tile([OUT, ISL], FP32, tag="fin_m")
        nc.vector.tensor_scalar(m[:, :], o_fin[:, :], 0.0, None, ALU.min)
        e = b.misc.tile([OUT, ISL], FP32, tag="fin_e")
        nc.scalar.activation(e[:, :], m[:, :], AF.Exp)
        r = b.misc.tile([OUT, ISL], FP32, tag="fin_r")
        nc.vector.tensor_scalar(r[:, :], o_fin[:, :], 0.0, -1.0, ALU.max,
                                ALU.add)
        fin = b.misc.tile([OUT, ISL], FP32, tag="fin")
        nc.vector.tensor_add(fin[:, :], e[:, :], r[:, :])
        nc.sync.dma_start(outT[:, :], fin[:, :])
    nc.compile()
    return nc


def make_in_maps(inputs):
    x = np.asarray(inputs["x"], np.float32)
    adj = np.asarray(inputs["adj"], np.float32)
    observation = np.asarray(inputs["observation"])
    theta = np.asarray(inputs["theta"], np.float32)
    W0 = np.asarray(inputs["W0"], np.float32)
    a0 = np.asarray(inputs["a0"], np.float32)
    W1 = np.asarray(inputs["W1"], np.float32)
    a1 = np.asarray(inputs["a1"], np.float32)
    Wo = np.asarray(inputs["Wo"], np.float32)
    ao = np.asarray(inputs["ao"], np.float32)

    bf = ml_dtypes.bfloat16
    f8 = ml_dtypes.float8_e4m3
    seed = (observation[0] == 1).astype(np.float32)[:, None]
    h = x + seed * theta                     # mergeState folded host-side
    hT = np.ascontiguousarray(h.T).astype(bf)
    adjT = np.ascontiguousarray((adj > 0).T.astype(np.float32)).astype(f8)
    wo_bf = Wo.astype(bf)

    in_maps = []
    for c in range(NCORES):
        psel_f = np.zeros((N, ISL), np.float32)
        psel_f[c * ISL + np.arange(ISL), np.arange(ISL)] = 1.0
        psel_c = np.concatenate(
            [np.concatenate([psel_f[2 * p * 128 : (2 * p + 1) * 128],
                             psel_f[(2 * p + 1) * 128 : (2 * p + 2) * 128]],
                            axis=1) for p in range(NJB // 2)], axis=0)
        in_maps.append({
            "psel": psel_c.astype(f8),
            "xT": hT,
            "adjT": adjT,
            "adjT_osl": np.ascontiguousarray(adjT[:, c * ISL : (c + 1) * ISL]),
            "w0": W0[c].astype(bf),
            "a01": (W0[c] @ a0[c][:D]).astype(bf),
            "a02": (W0[c] @ a0[c][D:]).astype(bf),
            "w1": W1[c].astype(bf),
            "a11": (W1[c] @ a1[c][:D]).astype(bf),
            "a12": (W1[c] @ a1[c][D:]).astype(bf),
            "wo": np.ascontiguousarray(wo_bf[c * D : (c + 1) * D]),
            "wo1": (Wo[c * D : (c + 1) * D] @ ao[:OUT]).astype(bf),
            "wo2": (Wo[c * D : (c + 1) * D] @ ao[OUT:]).astype(bf),
        })
    return in_maps


def kernel(**inputs):
    in_maps = make_in_maps(inputs)
    nc = build()
    res = run_bass_kernel_spmd(nc, in_maps, core_ids=list(range(NCORES)))
    out = np.concatenate(
        [res.results[c]["outT"].T for c in range(NCORES)], axis=0
    )
    return np.ascontiguousarray(out, np.float32)


if __name__ == "__main__":
    build()
    print("built ok")
